# revision 1
# baseline (speedup 1.0000x reference)
"""Trainium2 Bass kernel for nn_Attention_326417514823.

Per-batch computation (B=8, N=2048, D=256), one batch per NeuronCore:
    S = Q @ K.T / sqrt(D)                  (N x N)
    S[q, :] = -1e9 where mask[q] == 0      (row masking by query index)
    A = softmax(S, axis=0)                 (normalize over q, per column k)
    A[q, :] = 0 where mask[q] == 0
    O = A @ V                              (N x D)

Key algebra used on device: the softmax normalizer c[k] = sum_q E[q,k] is
per-column, so it can be folded into V (W[k,:] = V[k,:] / c[k]) and the
output becomes O = E @ W with E = exp(S/16) * mask[q].  No max-subtraction
is needed: scores/16 stay in [-7, 7], and the reference's masked entries
are exp(-1e9 - max) == 0 exactly in fp32, which matching the mask-multiply
reproduces bit-for-bit (zero).

Device layout (everything transposed so softmax reduces along the free axis
and both matmuls need no on-chip transpose):
    ST[k, q] = KT.T @ QT   with KT = K.T, QT = Q.T  (d on partitions)
    E[k, q]  = exp(ST/16) * mask_bcast               (bf16)
    c[k]     = sum_q E[k, q]   (fused accumulate in the mask multiply)
    W[k, :]  = V[k, :] * (1/c[k])                    (bf16)
    OT[d, q] = sum_k W[k,d] * E[k,q]   (PSUM accumulation over k-blocks)
Host transposes OT back to O.
"""

import numpy as np
import ml_dtypes

B, N, D = 8, 2048, 256
NCORES = 8
P = 128          # partitions
MMN = 512        # matmul moving free dim
KB = N // P      # 16 k-blocks
QC = N // MMN    # 4 q-chunks
DT = D // P      # 2 d-tiles

_cached = None


def _build():
    import concourse.bacc as bacc
    import concourse.mybir as mybir
    import concourse.tile as tile

    f32 = mybir.dt.float32
    bf16 = mybir.dt.bfloat16
    MULT = mybir.AluOpType.mult
    EXP = mybir.ActivationFunctionType.Exp

    nc = bacc.Bacc()
    kt = nc.dram_tensor("kt", [D, N], bf16, kind="ExternalInput")
    qt = nc.dram_tensor("qt", [D, N], bf16, kind="ExternalInput")
    v = nc.dram_tensor("v", [N, D], f32, kind="ExternalInput")
    mb = nc.dram_tensor("mb", [1, N], bf16, kind="ExternalInput")
    ot = nc.dram_tensor("ot", [D, N], f32, kind="ExternalOutput")

    with tile.TileContext(nc) as tc:
        with (
            tc.tile_pool(name="const", bufs=1) as constp,
            tc.tile_pool(name="epool", bufs=1) as epool,
            tc.tile_pool(name="wpool", bufs=1) as wpool,
            tc.tile_pool(name="vpool", bufs=3) as vpool,
            tc.tile_pool(name="cpool", bufs=3) as cpool,
            tc.tile_pool(name="etmp", bufs=3) as etmp,
            tc.tile_pool(name="outp", bufs=3) as outp,
        ):
            kt_sb, qt_sb = [], []
            for d in range(DT):
                kts = constp.tile([P, N], bf16, name=f"kts{d}")
                nc.sync.dma_start(kts[:], kt[d * P:(d + 1) * P, :])
                kt_sb.append(kts)
                qts = constp.tile([P, N], bf16, name=f"qts{d}")
                nc.sync.dma_start(qts[:], qt[d * P:(d + 1) * P, :])
                qt_sb.append(qts)
            mrow = constp.tile([1, N], bf16, name="mrow")
            nc.sync.dma_start(mrow[:], mb[:])
            mbc = constp.tile([P, N], bf16, name="mbc")
            nc.gpsimd.partition_broadcast(mbc[:], mrow[:])

            e_all, w_all = [], []
            with tc.tile_pool(name="ps1", bufs=2, space="PSUM") as ps1:
                for kb in range(KB):
                    st = ps1.tile([P, N], f32, name="st")
                    for d in range(DT):
                        for qc in range(QC):
                            nc.tensor.matmul(
                                st[:, qc * MMN:(qc + 1) * MMN],
                                kt_sb[d][:, kb * P:(kb + 1) * P],
                                qt_sb[d][:, qc * MMN:(qc + 1) * MMN],
                                start=(d == 0),
                                stop=(d == DT - 1),
                            )
                    eraw = etmp.tile([P, N], bf16, name="eraw")
                    nc.scalar.activation(eraw[:], st[:], EXP, scale=1.0 / 16.0)
                    e_kb = epool.tile([P, N], bf16, name=f"e{kb}")
                    c_kb = cpool.tile([P, 1], f32, name="c")
                    nc.vector.scalar_tensor_tensor(
                        e_kb[:], eraw[:], 1.0, mbc[:], MULT, MULT,
                        accum_out=c_kb[:],
                    )
                    rc = cpool.tile([P, 1], f32, name="rc")
                    nc.vector.reciprocal(rc[:], c_kb[:])
                    v_kb = vpool.tile([P, D], f32, name="v_t")
                    nc.sync.dma_start(v_kb[:], v[kb * P:(kb + 1) * P, :])
                    w_kb = wpool.tile([P, D], bf16, name=f"w{kb}")
                    nc.vector.tensor_scalar_mul(w_kb[:], v_kb[:], rc[:])
                    e_all.append(e_kb)
                    w_all.append(w_kb)

            with tc.tile_pool(name="ps2", bufs=1, space="PSUM") as ps2:
                otps = [
                    [ps2.tile([P, MMN], f32, name=f"otp{dh}_{qc}")
                     for qc in range(QC)]
                    for dh in range(DT)
                ]
                for kb in range(KB):
                    for dh in range(DT):
                        for qc in range(QC):
                            nc.tensor.matmul(
                                otps[dh][qc][:],
                                w_all[kb][:, dh * P:(dh + 1) * P],
                                e_all[kb][:, qc * MMN:(qc + 1) * MMN],
                                start=(kb == 0),
                                stop=(kb == KB - 1),
                            )
                for dh in range(DT):
                    for qc in range(QC):
                        ots = outp.tile([P, MMN], f32, name="ots")
                        nc.vector.tensor_copy(ots[:], otps[dh][qc][:])
                        nc.sync.dma_start(
                            ot[dh * P:(dh + 1) * P, qc * MMN:(qc + 1) * MMN],
                            ots[:],
                        )

    nc.compile()
    return nc


def _get_nc():
    global _cached
    if _cached is None:
        _cached = _build()
    return _cached


def kernel(key, query, value, mask, _trace=False):
    from concourse.bass_utils import run_bass_kernel_spmd

    nc = _get_nc()
    bf = ml_dtypes.bfloat16
    key = np.asarray(key, dtype=np.float32)
    query = np.asarray(query, dtype=np.float32)
    value = np.asarray(value, dtype=np.float32)
    mask = np.asarray(mask)

    in_maps = []
    for b in range(B):
        in_maps.append({
            "kt": np.ascontiguousarray(key[b].T).astype(bf),
            "qt": np.ascontiguousarray(query[b].T).astype(bf),
            "v": np.ascontiguousarray(value[b]),
            "mb": np.ascontiguousarray(mask[b]).astype(bf),
        })
    res = run_bass_kernel_spmd(
        nc, in_maps, core_ids=list(range(NCORES)), trace=_trace,
    )
    out = np.empty((B, N, D), np.float32)
    for b in range(B):
        out[b] = res.results[b]["ot"].T
    if _trace:
        return out, res
    return out


# revision 2
# speedup vs baseline: 1.1225x; 1.1225x over previous
"""Trainium2 Bass kernel for nn_Attention_326417514823.

Per-batch computation (B=8, N=2048, D=256), one batch per NeuronCore:
    S = Q @ K.T / sqrt(D)                  (N x N)
    S[q, :] = -1e9 where mask[q] == 0      (row masking by query index)
    A = softmax(S, axis=0)                 (normalize over q, per column k)
    A[q, :] = 0 where mask[q] == 0
    O = A @ V                              (N x D)

Algebra used on device: the softmax normalizer c[k] = sum_q E[q,k] is
per-column, so it folds into V (W[k,:] = V[k,:] / c[k]) and O = E @ W with
E = exp(S/16) * mask[q].  No max-subtraction is needed: scores/16 stay in
[-7, 7], and the reference's masked entries are exp(-1e9 - max) == 0
exactly in fp32, which the mask-multiply reproduces exactly (zero).

Device layout (transposed so the softmax reduction runs along the free axis
and neither matmul needs an on-chip transpose):
    ST[k, q] = KT.T @ QT   (KT = K.T, QT = Q.T, d on partitions)
    E[k, q]  = exp(ST/16) * mask_bcast              (bf16)
    c[k]     = sum_q E[k, q]  (fused accum in the DVE mask multiply)
    W[k, :]  = V[k, :] * (1/c[k])                   (bf16)
    OT[d, q] = sum_k W[k,d] * E[k,q]  (PSUM accumulation over k-blocks)
Host transposes OT back to O.

Pipelining: PSUM = 8 banks. 4 banks hold the q<1024 half of OT's
accumulators for the WHOLE kernel, so half of matmul-2 interleaves into
phase 1 (lagging 2 k-blocks behind the softmax pipeline). The score tiles
double-buffer in the other 4 banks; once phase 1 ends those 4 banks are
reused for the q>=1024 accumulators.
"""

import numpy as np
import ml_dtypes

B, N, D = 8, 2048, 256
NCORES = 8
P = 128          # partitions
MMN = 512        # matmul moving free dim (one PSUM bank of fp32)
KB = N // P      # 16 k-blocks
NCH = N // MMN   # 4 512-chunks along q
DT = D // P      # 2 d-tiles
LAG = 2          # k-blocks of slack before interleaved matmul-2 consumes W

_cached = None


def _build():
    import concourse.bacc as bacc
    import concourse.mybir as mybir
    import concourse.tile as tile

    f32 = mybir.dt.float32
    bf16 = mybir.dt.bfloat16
    MULT = mybir.AluOpType.mult
    EXP = mybir.ActivationFunctionType.Exp

    nc = bacc.Bacc()
    kt = nc.dram_tensor("kt", [D, N], bf16, kind="ExternalInput")
    qt = nc.dram_tensor("qt", [D, N], bf16, kind="ExternalInput")
    v = nc.dram_tensor("v", [N, D], f32, kind="ExternalInput")
    mb = nc.dram_tensor("mb", [1, N], bf16, kind="ExternalInput")
    ot = nc.dram_tensor("ot", [D, N], f32, kind="ExternalOutput")

    with tile.TileContext(nc) as tc:
        with (
            tc.tile_pool(name="const", bufs=1) as constp,
            tc.tile_pool(name="epool", bufs=1) as epool,
            tc.tile_pool(name="wpool", bufs=1) as wpool,
            tc.tile_pool(name="vpool", bufs=3) as vpool,
            tc.tile_pool(name="cpool", bufs=3) as cpool,
            tc.tile_pool(name="outp", bufs=4) as outp,
            # q<1024 OT accumulators live for the whole kernel (banks 0-3)
            tc.tile_pool(name="psA", bufs=1, space="PSUM") as psA,
        ):
            # inputs, chunked so the first matmuls start after ~128KB of DMA
            kt_ch = [[constp.tile([P, MMN], bf16, name=f"ktc{d}_{j}")
                      for j in range(NCH)] for d in range(DT)]
            qt_ch = [[constp.tile([P, MMN], bf16, name=f"qtc{d}_{j}")
                      for j in range(NCH)] for d in range(DT)]
            for d in range(DT):
                for j in range(NCH):
                    nc.sync.dma_start(
                        kt_ch[d][j][:], kt[d * P:(d + 1) * P, j * MMN:(j + 1) * MMN])
                    nc.sync.dma_start(
                        qt_ch[d][j][:], qt[d * P:(d + 1) * P, j * MMN:(j + 1) * MMN])
            mbc = constp.tile([P, N], bf16, name="mbc")
            nc.sync.dma_start(mbc[:], mb[0:1, :].partition_broadcast(P))

            accA = [[psA.tile([P, MMN], f32, name=f"accA{dh}_{qc}")
                     for qc in range(2)] for dh in range(DT)]

            e_all = [None] * KB
            w_all = [None] * KB

            def mm2(acc, kb, dh, qci):
                nc.tensor.matmul(
                    acc[:],
                    w_all[kb][:, dh * P:(dh + 1) * P],
                    e_all[kb][:, qci * MMN:(qci + 1) * MMN],
                    start=(kb == 0),
                    stop=(kb == KB - 1),
                )

            with tc.tile_pool(name="psS", bufs=2, space="PSUM") as psS:
                for kb in range(KB):
                    e_kb = epool.tile([P, N], bf16, name=f"e{kb}")
                    for ch in range(2):
                        st = psS.tile([P, 2 * MMN], f32, name="st")
                        for d in range(DT):
                            for s in range(2):
                                nc.tensor.matmul(
                                    st[:, s * MMN:(s + 1) * MMN],
                                    kt_ch[d][kb // 4][:, (kb % 4) * P:(kb % 4 + 1) * P],
                                    qt_ch[d][ch * 2 + s][:],
                                    start=(d == 0),
                                    stop=(d == DT - 1),
                                )
                        nc.scalar.activation(
                            e_kb[:, ch * 2 * MMN:(ch + 1) * 2 * MMN], st[:],
                            EXP, scale=1.0 / 16.0)
                    c_kb = cpool.tile([P, 1], f32, name="c")
                    nc.vector.scalar_tensor_tensor(
                        e_kb[:], e_kb[:], 1.0, mbc[:], MULT, MULT,
                        accum_out=c_kb[:])
                    rc = cpool.tile([P, 1], f32, name="rc")
                    nc.vector.reciprocal(rc[:], c_kb[:])
                    v_kb = vpool.tile([P, D], f32, name="v_t")
                    nc.sync.dma_start(v_kb[:], v[kb * P:(kb + 1) * P, :])
                    w_kb = wpool.tile([P, D], bf16, name=f"w{kb}")
                    nc.vector.tensor_scalar_mul(w_kb[:], v_kb[:], rc[:])
                    e_all[kb] = e_kb
                    w_all[kb] = w_kb

                    # interleaved half of matmul-2, LAG k-blocks behind
                    if kb >= LAG:
                        for dh in range(DT):
                            for qci in range(2):
                                mm2(accA[dh][qci], kb - LAG, dh, qci)
                for j in range(KB - LAG, KB):
                    for dh in range(DT):
                        for qci in range(2):
                            mm2(accA[dh][qci], j, dh, qci)

            # q<1024 results: copy + store (overlaps the q>=1024 matmuls)
            def store(acc, dh, qci, engine):
                o_sb = outp.tile([P, MMN], f32, name="o_sb")
                if engine == "act":
                    nc.scalar.copy(o_sb[:], acc[:])
                else:
                    nc.vector.tensor_copy(o_sb[:], acc[:])
                nc.sync.dma_start(
                    ot[dh * P:(dh + 1) * P, qci * MMN:(qci + 1) * MMN], o_sb[:])

            with tc.tile_pool(name="psB", bufs=1, space="PSUM") as psB:
                accB = [[psB.tile([P, MMN], f32, name=f"accB{dh}_{qc}")
                         for qc in range(2)] for dh in range(DT)]
                for dh in range(DT):
                    for qci in range(2):
                        store(accA[dh][qci], dh, qci, "act" if dh == 0 else "dve")
                for kb in range(KB):
                    for dh in range(DT):
                        for qci in range(2, NCH):
                            mm2(accB[dh][qci - 2], kb, dh, qci)
                for dh in range(DT):
                    for qci in range(2, NCH):
                        store(accB[dh][qci - 2], dh, qci,
                              "act" if dh == 0 else "dve")

    nc.compile()
    return nc


def _get_nc():
    global _cached
    if _cached is None:
        _cached = _build()
    return _cached


def kernel(key, query, value, mask):
    from concourse.bass_utils import run_bass_kernel_spmd

    nc = _get_nc()
    bf = ml_dtypes.bfloat16
    key = np.asarray(key, dtype=np.float32)
    query = np.asarray(query, dtype=np.float32)
    value = np.asarray(value, dtype=np.float32)
    mask = np.asarray(mask)

    in_maps = []
    for b in range(B):
        in_maps.append({
            "kt": np.ascontiguousarray(key[b].T).astype(bf),
            "qt": np.ascontiguousarray(query[b].T).astype(bf),
            "v": np.ascontiguousarray(value[b]),
            "mb": np.ascontiguousarray(mask[b]).astype(bf),
        })
    res = run_bass_kernel_spmd(nc, in_maps, core_ids=list(range(NCORES)))
    out = np.empty((B, N, D), np.float32)
    for b in range(B):
        out[b] = res.results[b]["ot"].T
    return out


# revision 4
# speedup vs baseline: 1.1641x; 1.0371x over previous
"""Trainium2 Bass kernel for nn_Attention_326417514823.

Per-batch computation (B=8, N=2048, D=256), one batch per NeuronCore:
    S = Q @ K.T / sqrt(D)                  (N x N)
    S[q, :] = -1e9 where mask[q] == 0      (row masking by query index)
    A = softmax(S, axis=0)                 (normalize over q, per column k)
    A[q, :] = 0 where mask[q] == 0
    O = A @ V                              (N x D)

Algebra used on device: the softmax normalizer c[k] = sum_q E[q,k] is
per-column, so it folds into V (W[k,:] = V[k,:] / c[k]) and O = E @ W with
E = exp(S/16) * mask[q].  No max-subtraction is needed: scores/16 stay in
[-7, 7], and the reference's masked entries are exp(-1e9 - max) == 0
exactly in fp32, which the mask-multiply reproduces exactly (zero).

Device layout (transposed so the softmax reduction runs along the free axis
and neither matmul needs an on-chip transpose):
    ST[k, q] = KT.T @ QT   (KT = K.T, QT = Q.T, d on partitions)
    E[k, q]  = exp(ST/16) * mask_bcast              (bf16)
    c[k]     = sum_q E[k, q]  (fused accum in the DVE mask multiply)
    W[k, :]  = V[k, :] * (1/c[k])                   (bf16)
    OT[d, q] = sum_k W[k,d] * E[k,q]  (PSUM accumulation over k-blocks)
Host transposes OT back to O.

Pipelining: PSUM = 8 banks. 4 banks hold the q<1024 half of OT's
accumulators for the WHOLE kernel, so half of matmul-2 interleaves into
phase 1 (lagging 2 k-blocks behind the softmax pipeline). The score tiles
double-buffer in the other 4 banks; once phase 1 ends those 4 banks are
reused for the q>=1024 accumulators.
"""

import numpy as np
import ml_dtypes

B, N, D = 8, 2048, 256
NCORES = 8
P = 128          # partitions
MMN = 512        # matmul moving free dim (one PSUM bank of fp32)
KB = N // P      # 16 k-blocks
NCH = N // MMN   # 4 512-chunks along q
DT = D // P      # 2 d-tiles
LAG = 2          # k-blocks of slack before interleaved matmul-2 consumes W

_cached = None


def _build():
    import concourse.bacc as bacc
    import concourse.mybir as mybir
    import concourse.tile as tile

    f32 = mybir.dt.float32
    bf16 = mybir.dt.bfloat16
    MULT = mybir.AluOpType.mult
    EXP = mybir.ActivationFunctionType.Exp

    nc = bacc.Bacc()
    kt = nc.dram_tensor("kt", [D, N], bf16, kind="ExternalInput")
    qt = nc.dram_tensor("qt", [D, N], bf16, kind="ExternalInput")
    v = nc.dram_tensor("v", [N, D], f32, kind="ExternalInput")
    mb = nc.dram_tensor("mb", [1, N], bf16, kind="ExternalInput")
    ot = nc.dram_tensor("ot", [D, N], f32, kind="ExternalOutput")

    with tile.TileContext(nc) as tc:
        with (
            tc.tile_pool(name="const", bufs=1) as constp,
            tc.tile_pool(name="epool", bufs=1) as epool,
            tc.tile_pool(name="wpool", bufs=1) as wpool,
            tc.tile_pool(name="vpool", bufs=3) as vpool,
            tc.tile_pool(name="cpool", bufs=3) as cpool,
            tc.tile_pool(name="outp", bufs=4) as outp,
            # q<1024 OT accumulators live for the whole kernel (banks 0-3)
            tc.tile_pool(name="psA", bufs=1, space="PSUM") as psA,
        ):
            # inputs, chunked so the first matmuls start after ~128KB of DMA
            kt_ch = [[constp.tile([P, MMN], bf16, name=f"ktc{d}_{j}")
                      for j in range(NCH)] for d in range(DT)]
            qt_ch = [[constp.tile([P, MMN], bf16, name=f"qtc{d}_{j}")
                      for j in range(NCH)] for d in range(DT)]
            # first-needed chunks first: kb=0 uses kt[*][0] and all qt chunks
            for j in range(NCH):
                for d in range(DT):
                    nc.sync.dma_start(
                        qt_ch[d][j][:], qt[d * P:(d + 1) * P, j * MMN:(j + 1) * MMN])
                    nc.sync.dma_start(
                        kt_ch[d][j][:], kt[d * P:(d + 1) * P, j * MMN:(j + 1) * MMN])
            mbc = constp.tile([P, N], bf16, name="mbc")
            nc.sync.dma_start(mbc[:], mb[0:1, :].partition_broadcast(P))

            accA = [[psA.tile([P, MMN], f32, name=f"accA{dh}_{qc}")
                     for qc in range(2)] for dh in range(DT)]

            e_all = [None] * KB
            w_all = [None] * KB

            def mm2(acc, kb, dh, qci):
                nc.tensor.matmul(
                    acc[:],
                    w_all[kb][:, dh * P:(dh + 1) * P],
                    e_all[kb][:, qci * MMN:(qci + 1) * MMN],
                    start=(kb == 0),
                    stop=(kb == KB - 1),
                )

            with tc.tile_pool(name="psS", bufs=2, space="PSUM") as psS:
                for kb in range(KB):
                    e_kb = epool.tile([P, N], bf16, name=f"e{kb}")
                    for ch in range(2):
                        st = psS.tile([P, 2 * MMN], f32, name="st")
                        for d in range(DT):
                            for s in range(2):
                                nc.tensor.matmul(
                                    st[:, s * MMN:(s + 1) * MMN],
                                    kt_ch[d][kb // 4][:, (kb % 4) * P:(kb % 4 + 1) * P],
                                    qt_ch[d][ch * 2 + s][:],
                                    start=(d == 0),
                                    stop=(d == DT - 1),
                                )
                        nc.scalar.activation(
                            e_kb[:, ch * 2 * MMN:(ch + 1) * 2 * MMN], st[:],
                            EXP, scale=1.0 / 16.0)
                    c_kb = cpool.tile([P, 1], f32, name="c")
                    nc.vector.scalar_tensor_tensor(
                        e_kb[:], e_kb[:], 1.0, mbc[:], MULT, MULT,
                        accum_out=c_kb[:])
                    rc = cpool.tile([P, 1], f32, name="rc")
                    nc.vector.reciprocal(rc[:], c_kb[:])
                    v_kb = vpool.tile([P, D], f32, name="v_t")
                    nc.sync.dma_start(v_kb[:], v[kb * P:(kb + 1) * P, :])
                    w_kb = wpool.tile([P, D], bf16, name=f"w{kb}")
                    nc.vector.tensor_scalar_mul(w_kb[:], v_kb[:], rc[:])
                    e_all[kb] = e_kb
                    w_all[kb] = w_kb

                    # interleaved half of matmul-2, LAG k-blocks behind
                    if kb >= LAG:
                        for dh in range(DT):
                            for qci in range(2):
                                mm2(accA[dh][qci], kb - LAG, dh, qci)
                for j in range(KB - LAG, KB):
                    for dh in range(DT):
                        for qci in range(2):
                            mm2(accA[dh][qci], j, dh, qci)

            # q<1024 results: copy + store (overlaps the q>=1024 matmuls)
            def store(acc, dh, qci, engine):
                o_sb = outp.tile([P, MMN], f32, name="o_sb")
                if engine == "act":
                    nc.scalar.copy(o_sb[:], acc[:])
                else:
                    nc.vector.tensor_copy(o_sb[:], acc[:])
                nc.sync.dma_start(
                    ot[dh * P:(dh + 1) * P, qci * MMN:(qci + 1) * MMN], o_sb[:])

            with tc.tile_pool(name="psB", bufs=1, space="PSUM") as psB:
                accB = [[psB.tile([P, MMN], f32, name=f"accB{dh}_{qc}")
                         for qc in range(2)] for dh in range(DT)]
                for dh in range(DT):
                    for qci in range(2):
                        store(accA[dh][qci], dh, qci, "act" if dh == 0 else "dve")
                # chain-per-accumulator so each finishes early and its copy
                # overlaps the remaining accumulation chains
                for qci in range(2, NCH):
                    for dh in range(DT):
                        for kb in range(KB):
                            mm2(accB[dh][qci - 2], kb, dh, qci)
                        store(accB[dh][qci - 2], dh, qci,
                              "act" if dh == 0 else "dve")

    nc.compile()
    return nc


def _get_nc():
    global _cached
    if _cached is None:
        _cached = _build()
    return _cached


def kernel(key, query, value, mask):
    from concourse.bass_utils import run_bass_kernel_spmd

    nc = _get_nc()
    bf = ml_dtypes.bfloat16
    key = np.asarray(key, dtype=np.float32)
    query = np.asarray(query, dtype=np.float32)
    value = np.asarray(value, dtype=np.float32)
    mask = np.asarray(mask)

    in_maps = []
    for b in range(B):
        in_maps.append({
            "kt": np.ascontiguousarray(key[b].T).astype(bf),
            "qt": np.ascontiguousarray(query[b].T).astype(bf),
            "v": np.ascontiguousarray(value[b]),
            "mb": np.ascontiguousarray(mask[b]).astype(bf),
        })
    res = run_bass_kernel_spmd(nc, in_maps, core_ids=list(range(NCORES)))
    out = np.empty((B, N, D), np.float32)
    for b in range(B):
        out[b] = res.results[b]["ot"].T
    return out
